# revision 8
# baseline (speedup 1.0000x reference)
"""EntMaxSelectLayer distributed Trainium2 kernel (v3).

Computes out = x @ entmax15(weight, axis=-1) with
  x [512, 8192] f32, weight [8192, 4096] f32, out [512, 4096] f32.

Strategy (8 NeuronCores, SPMD, f16 on-chip; rel err ~6.5e-3, gate 2e-2):
  - weight row-sharded: core d gets rows [1024d, 1024d+1024) as f16.
    All 8 [128,4096] tiles are DMA-prefetched up-front (sync queue runs
    ~340GB/s across 16 engines; deep wpool so triggers never wait).
  - per-tile entmax front: top-8-of-each-512-chunk candidates (DVE max8,
    the irreducible scan), top-24 sort rounds (DVE), threshold recursion
    spread over GpSimd (small tensor_tensor chain) + DVE (scans) + Act
    (sqrt). Back phase: tau -> cneg, relu on Act, squares split across
    DVE/GpSimd/Act to balance engine load. p_t lands every ~6us.
  - matmul in 2 sessions per (b,kq) group: s1 = tiles 0-3 accumulated in
    PSUM during the ramp and evacuated to f16 SBUF, s2 = tiles 4-7 after
    p_7 merged with s1 via scalar_tensor_tensor. This lets ~50% of PE
    work complete while p tiles are still being produced (PSUM can only
    hold 1/4 of the output at once).
  - b-major session order: batch-block partials complete in sequence,
    each [128,4096] f16 partial DMAs to DRAM and ReduceScatter(add)s
    across the 8 cores as soon as it is ready; RS output [16,4096] f16
    copies straight to the f16 external output (host converts to f32).
    A tiny AllGather barrier up-front absorbs NRT startup skew.
"""

import numpy as np

B, IN, OUT = 512, 8192, 4096
NCORES = 8
ROWS = IN // NCORES          # 1024 weight rows per core
NT = ROWS // 128             # 8 weight tiles of [128, 4096] per core
T = 24                       # sorted prefix for the mini-entmax
NCAND = 64                   # candidate count (top-8 of each 512-chunk)
NEG_FILL = -60000.0          # f16-safe "minus infinity" for match_replace
NB = B // 128                # 4 batch blocks
S1_TILES = 4                 # session 1 = tiles 0..3, session 2 = 4..7

_cache = {}


def _build_program():
    from concourse import bacc, mybir, tile
    from concourse.alu_op_type import AluOpType

    f32 = mybir.dt.float32
    f16 = mybir.dt.float16

    nc = bacc.Bacc(
        "TRN2",
        target_bir_lowering=False,
        debug=False,
        enable_asserts=False,
        num_devices=NCORES,
    )

    w_ext = nc.dram_tensor("w", [ROWS, OUT], f16, kind="ExternalInput")
    # host pre-tiles xT: xT[p, t*512 + b*128 + j] = x[b*128+j, 1024d+128t+p]*0.25
    xT_ext = nc.dram_tensor("xT", [128, NT * B], f16, kind="ExternalInput")
    consts_ext = nc.dram_tensor("consts", [128, 2 * T], f32, kind="ExternalInput")
    # f16 output: row-chunk b holds this core's [16,4096] slice of batch
    # block b; host converts to f32 and scatters.
    out_ext = nc.dram_tensor("out", [4 * 16, OUT], f16, kind="ExternalOutput")

    rg = [list(range(NCORES))]

    with tile.TileContext(nc) as tc:
        with (
            tc.tile_pool(name="consts", bufs=1) as cpool,
            tc.tile_pool(name="wpool", bufs=6) as wpool,
            tc.tile_pool(name="ppool", bufs=NT) as ppool,
            tc.tile_pool(name="xpool", bufs=1) as xpool,
            tc.tile_pool(name="small", bufs=2) as spool,
            tc.tile_pool(name="s1part", bufs=NB) as s1pool,
            tc.tile_pool(name="mrg", bufs=2) as mpool,
            tc.tile_pool(name="psum", bufs=8, space="PSUM") as psum_pool,
            tc.tile_pool(name="dram", bufs=1, space="DRAM") as dpool,
        ):
            # ---- constants (host-provided: [:, :T]=1/rho, [:, T:]=0) ----
            cst = cpool.tile([128, 2 * T], f32, name="cst")
            nc.scalar.dma_start(out=cst[:], in_=consts_ext.ap())

            # tiny barrier collective up-front: absorbs NRT startup stagger
            bar_in = dpool.tile([1, 64], f16, name="bar_in")
            bar_out = dpool.tile([8, 64], f16, name="bar_out")
            nc.gpsimd.collective_compute(
                "AllGather",
                mybir.AluOpType.bypass,
                replica_groups=rg,
                ins=[bar_in.opt()],
                outs=[bar_out.opt()],
            )
            rinv = cst[:, 0:T]
            zero64 = cst[:, T:2 * T]

            stash = {}
            p_tiles = []

            # weight prefetch: wt0, xT, wt1..wt7 all queued immediately
            def load_wt(t):
                wt = wpool.tile([128, OUT], f16, name=f"wt{t}", tag="wt", bufs=6)
                nc.sync.dma_start(out=wt[:], in_=w_ext.ap()[128 * t:128 * (t + 1), :])
                stash[("wt", t)] = wt
                if t == 0:
                    xT_sb = xpool.tile([128, NT * B], f16, name="xT_sb")
                    nc.sync.dma_start(out=xT_sb[:], in_=xT_ext.ap())
                    stash["xT"] = xT_sb

            def front(t):
                wt = stash[("wt", t)]
                cand = spool.tile([128, NCAND], f16, tag="cand", bufs=2)
                for c in range(8):
                    nc.vector.max(cand[:, 8 * c:8 * c + 8], wt[:, 512 * c:512 * (c + 1)])
                v64 = spool.tile([128, T], f16, tag="v64", bufs=2)
                for j in range(T // 8):
                    nc.vector.max(v64[:, 8 * j:8 * j + 8], cand[:])
                    if j < T // 8 - 1:
                        nc.vector.match_replace(
                            cand[:], v64[:, 8 * j:8 * j + 8], cand[:], NEG_FILL
                        )
                m32 = spool.tile([128, 1], f32, tag="m32", bufs=2)
                nc.gpsimd.tensor_copy(m32[:], v64[:, 0:1])
                zs = spool.tile([128, T], f32, tag="zs", bufs=2)
                nc.vector.tensor_scalar(
                    zs[:], v64[:], m32[:], 0.5, AluOpType.subtract, AluOpType.mult
                )
                zsq = spool.tile([128, T], f32, tag="zsq", bufs=2)
                nc.gpsimd.tensor_tensor(zsq[:], zs[:], zs[:], AluOpType.mult)
                cs1 = spool.tile([128, T], f32, tag="cs1", bufs=2)
                nc.vector.tensor_tensor_scan(
                    cs1[:], zs[:], zero64, 0.0, AluOpType.add, AluOpType.add
                )
                cs2 = spool.tile([128, T], f32, tag="cs2", bufs=2)
                nc.vector.tensor_tensor_scan(
                    cs2[:], zsq[:], zero64, 0.0, AluOpType.add, AluOpType.add
                )
                mean = spool.tile([128, T], f32, tag="mean", bufs=2)
                nc.gpsimd.tensor_tensor(mean[:], cs1[:], rinv, AluOpType.mult)
                msq = spool.tile([128, T], f32, tag="msq", bufs=2)
                nc.gpsimd.tensor_tensor(msq[:], cs2[:], rinv, AluOpType.mult)
                ms2 = spool.tile([128, T], f32, tag="ms2", bufs=2)
                nc.gpsimd.tensor_tensor(ms2[:], mean[:], mean[:], AluOpType.mult)
                dta = spool.tile([128, T], f32, tag="dta", bufs=2)
                nc.gpsimd.tensor_tensor(dta[:], rinv, msq[:], AluOpType.subtract)
                nc.gpsimd.tensor_tensor(dta[:], dta[:], ms2[:], AluOpType.add)
                nc.gpsimd.tensor_single_scalar(dta[:], dta[:], 0.0, AluOpType.max)
                sq = spool.tile([128, T], f32, tag="sq", bufs=2)
                nc.scalar.activation(sq[:], dta[:], mybir.ActivationFunctionType.Sqrt)
                stash[t] = (wt, m32, zs, mean, sq)

            def back(t):
                wt, m32, zs, mean, sq = stash.pop(t)
                tau = spool.tile([128, T], f32, tag="tau", bufs=2)
                nc.gpsimd.tensor_tensor(tau[:], mean[:], sq[:], AluOpType.subtract)
                cond = spool.tile([128, T], f32, tag="cond", bufs=2)
                nc.vector.tensor_tensor(cond[:], tau[:], zs[:], AluOpType.is_le)
                tsel = spool.tile([128, T], f32, tag="tsel", bufs=2)
                nc.vector.scalar_tensor_tensor(
                    tsel[:], tau[:], 100.0, cond[:], AluOpType.add, AluOpType.mult
                )
                tmax = spool.tile([128, 1], f32, tag="tmax", bufs=2)
                nc.vector.tensor_reduce(
                    tmax[:], tsel[:], mybir.AxisListType.X, AluOpType.max
                )
                c1 = spool.tile([128, 1], f32, tag="c1", bufs=2)
                nc.vector.tensor_scalar(
                    c1[:], tmax[:], -2.0, 200.0, AluOpType.mult, AluOpType.add
                )
                cneg = spool.tile([128, 1], f32, tag="cneg", bufs=2)
                nc.gpsimd.tensor_tensor(cneg[:], c1[:], m32[:], AluOpType.subtract)
                r = spool.tile([128, OUT], f16, tag="r", bufs=2, name=f"r{t}")
                nc.scalar.activation(
                    r[:], wt[:], mybir.ActivationFunctionType.Relu,
                    bias=cneg[:], scale=1.0,
                )
                p = ppool.tile([128, OUT], f16, tag="p", name=f"p{t}")
                # squares split: DVE for t in {1,4}, GpSimd {2,6}, Act rest
                if t in (1, 4):
                    nc.vector.tensor_tensor(p[:], r[:], r[:], AluOpType.mult)
                elif t in (2, 6):
                    nc.gpsimd.tensor_tensor(p[:], r[:], r[:], AluOpType.mult)
                else:
                    nc.scalar.activation(
                        p[:], r[:], mybir.ActivationFunctionType.Square
                    )
                p_tiles.append(p)

            # ---- matmul sessions ----
            # session s of batch block b: psum[b,kq] += sum_{t in s} xT_t_b @ p_t_kq
            # s1 evacuates to f16 SBUF; s2 merges s1 + psum -> f16, DMAs out.
            def mm_session(b, t0, t1, ps_tiles):
                xT_sb = stash["xT"]
                for t in range(t0, t1):
                    for kq in range(8):
                        nc.tensor.matmul(
                            ps_tiles[kq][:],
                            lhsT=xT_sb[:, 512 * t + 128 * b:512 * t + 128 * (b + 1)],
                            rhs=p_tiles[t][:, 512 * kq:512 * (kq + 1)],
                            start=(t == t0),
                            stop=(t == t1 - 1),
                        )

            def evac_s1(b, ps_tiles, s1parts):
                # psum -> f16 SBUF partial for this b; spread over 3 engines
                s1p = s1pool.tile([128, OUT], f16, tag="s1p", name=f"s1p{b}")
                for kq in range(8):
                    dst = s1p[:, 512 * kq:512 * (kq + 1)]
                    if kq % 2 == 0:
                        nc.vector.tensor_copy(dst, ps_tiles[kq][:])
                    else:
                        nc.scalar.copy(dst, ps_tiles[kq][:])
                s1parts[b] = s1p

            def merge_s2(b, ps_tiles, s1parts, partial):
                # out16 = psum(s2) + s1 partial, f16; then DMA to DRAM partial
                s1p = s1parts[b]
                out16 = mpool.tile([128, OUT], f16, tag="mrg", bufs=2,
                                   name=f"mrg{b}")
                tmp = mpool.tile([128, OUT // 2], f16, tag="mtmp", bufs=2,
                                 name=f"mtmp{b}")
                # even kq: DVE merges psum+s1 directly; odd kq: Act copies
                # psum out, DVE adds the two f16 halves (2x-mode, cheap)
                for kq in range(1, 8, 2):
                    j = (kq - 1) // 2
                    nc.scalar.copy(tmp[:, 512 * j:512 * (j + 1)],
                                   ps_tiles[kq][:, 0:512])
                for kq in range(0, 8, 2):
                    nc.vector.scalar_tensor_tensor(
                        out16[:, 512 * kq:512 * (kq + 1)],
                        ps_tiles[kq][:], 1.0,
                        s1p[:, 512 * kq:512 * (kq + 1)],
                        AluOpType.mult, AluOpType.add,
                    )
                for kq in range(1, 8, 2):
                    j = (kq - 1) // 2
                    nc.vector.tensor_tensor(
                        out16[:, 512 * kq:512 * (kq + 1)],
                        tmp[:, 512 * j:512 * (j + 1)],
                        s1p[:, 512 * kq:512 * (kq + 1)],
                        AluOpType.add,
                    )
                nc.sync.dma_start(out=partial[:], in_=out16[:])

            rs_outs = []

            def rs_block(b, partial):
                rsout = dpool.tile([16, OUT], f16, name=f"rsout{b}")
                nc.gpsimd.collective_compute(
                    "ReduceScatter",
                    mybir.AluOpType.add,
                    replica_groups=rg,
                    ins=[partial.opt()],
                    outs=[rsout.opt()],
                )
                rs_outs.append((b, rsout))

            def rs_flush():
                b, rsout = rs_outs.pop(0)
                # direct DRAM->DRAM copy into the f16 output
                nc.scalar.dma_start(
                    out=out_ext.ap()[16 * b:16 * (b + 1), :], in_=rsout[:]
                )

            # ---------------- emission schedule ----------------
            for t in range(NT):
                load_wt(t)

            s1parts = {}
            partials = [dpool.tile([128, OUT], f16, name=f"partial{b}")
                        for b in range(NB)]
            ps_s1 = {}

            for t in range(NT):
                front(t)
                if t >= 1:
                    back(t - 1)
                # after p_3 exists (back(3) ran when t==4): emit s1 sessions
                # one per front step so evac1 ops land in matching exec slots
                if t >= 4:
                    b = t - 4
                    ps = [psum_pool.tile([128, 512], f32, tag="ps",
                                         name=f"ps1_{b}_{kq}")
                          for kq in range(8)]
                    ps_s1[b] = ps
                    mm_session(b, 0, S1_TILES, ps)
                    evac_s1(b, ps, s1parts)
            back(NT - 1)

            # s2 sessions + merge + DMA + RS, batch-block pipelined
            for b in range(NB):
                ps = [psum_pool.tile([128, 512], f32, tag="ps",
                                     name=f"ps2_{b}_{kq}")
                      for kq in range(8)]
                mm_session(b, S1_TILES, NT, ps)
                merge_s2(b, ps, s1parts, partials[b])
                rs_block(b, partials[b])
                if b >= 1:
                    rs_flush()
            rs_flush()

    nc.compile()
    return nc


def get_program():
    if "nc" not in _cache:
        _cache["nc"] = _build_program()
    return _cache["nc"]


def kernel(x: np.ndarray, weight: np.ndarray, trace: bool = False):
    from concourse.bass_utils import run_bass_kernel_spmd

    x = np.ascontiguousarray(x, dtype=np.float32)
    weight = np.ascontiguousarray(weight, dtype=np.float32)
    assert x.shape == (B, IN) and weight.shape == (IN, OUT)

    nc = get_program()
    in_maps = []
    for d in range(NCORES):
        wsh = np.ascontiguousarray(
            weight[ROWS * d:ROWS * (d + 1), :], dtype=np.float16
        )
        # xT[p, t*512 + b] = 0.25 * x[b, 1024d + 128t + p]
        xsh = (0.25 * x[:, ROWS * d:ROWS * (d + 1)].T).astype(np.float16)
        xt = np.ascontiguousarray(
            xsh.reshape(NT, 128, B).transpose(1, 0, 2).reshape(128, NT * B)
        )
        rho = np.arange(1, T + 1, dtype=np.float32)
        cst = np.zeros((128, 2 * T), dtype=np.float32)
        cst[:, 0:T] = 1.0 / rho
        in_maps.append({"w": wsh, "xT": xt, "consts": cst})
    res = run_bass_kernel_spmd(
        nc, in_maps, core_ids=list(range(NCORES)), trace=trace
    )
    out = np.empty((B, OUT), dtype=np.float32)
    for d in range(NCORES):
        o = res.results[d]["out"].astype(np.float32)  # [64, 4096]
        for b in range(NB):
            out[128 * b + 16 * d:128 * b + 16 * (d + 1), :] = \
                o[16 * b:16 * (b + 1), :]
    if trace:
        _cache["last_result"] = res
    return out


# revision 18
# speedup vs baseline: 1.1264x; 1.1264x over previous
"""EntMaxSelectLayer distributed Trainium2 kernel (v4).

Computes out = x @ entmax15(weight, axis=-1) with
  x [512, 8192] f32, weight [8192, 4096] f32, out [512, 4096] f32.

Strategy (8 NeuronCores, SPMD, f16 on-chip; rel err ~6.5e-3, gate 2e-2):
  - weight row-sharded: core d gets rows [1024d, 1024d+1024) as f16; all
    8 [128,4096] tiles DMA-prefetched up-front (sync queue, ~340GB/s).
  - per-tile entmax front: candidate scan via DVE max8 (the irreducible
    ~5.4us/tile), top-24 sort rounds (DVE), then the whole threshold
    recursion consolidated on GpSimd (plus one Act sqrt) to minimize
    cross-engine hops; relu on Act; squares split DVE/Act only (GpSimd
    big-op rate is ~2x worse). p_t cadence ~6us, p_7 ~60us.
  - matmul b-major: b0 and b1 use 2 sessions (tiles 0-3 evac'd to f16
    SBUF during the ramp, tiles 4-7 merged after p_7) so PE work overlaps
    the front and those blocks finish right after p_7; b2/b3 run full
    8-tile chains afterwards (PE is not the critical engine; the
    collective chain is).
  - reduction: per-block [128,4096] f16 partials DMA to DRAM and
    ReduceScatter(add) over 8 cores in 3 chunks (b0 | b1 | b2+b3) so the
    collective engine starts right after p_7 and stays busy; RS outputs
    copy DRAM->DRAM into the f16 external output (host converts to f32).
    A tiny AllGather barrier up-front absorbs NRT startup skew.
"""

import numpy as np

B, IN, OUT = 512, 8192, 4096
NCORES = 8
ROWS = IN // NCORES          # 1024 weight rows per core
NT = ROWS // 128             # 8 weight tiles of [128, 4096] per core
T = 24                       # sorted prefix for the mini-entmax
NCAND = 64                   # candidate count (top-8 of each 512-chunk)
NEG_FILL = -60000.0          # f16-safe "minus infinity" for match_replace
NB = B // 128                # 4 batch blocks
S1_TILES = 4                 # session 1 = tiles 0..3 (for b0/b1)

_cache = {}


def _build_program():
    from concourse import bacc, mybir, tile
    from concourse.alu_op_type import AluOpType

    f32 = mybir.dt.float32
    f16 = mybir.dt.float16

    nc = bacc.Bacc(
        "TRN2",
        target_bir_lowering=False,
        debug=False,
        enable_asserts=False,
        num_devices=NCORES,
    )

    w_ext = nc.dram_tensor("w", [ROWS, OUT], f16, kind="ExternalInput")
    xT_ext = nc.dram_tensor("xT", [128, NT * B], f16, kind="ExternalInput")
    consts_ext = nc.dram_tensor("consts", [128, 2 * T], f32, kind="ExternalInput")
    out_ext = nc.dram_tensor("out", [4 * 16, OUT], f16, kind="ExternalOutput")

    rg = [list(range(NCORES))]

    with tile.TileContext(nc) as tc:
        with (
            tc.tile_pool(name="consts", bufs=1) as cpool,
            tc.tile_pool(name="wpool", bufs=6) as wpool,
            tc.tile_pool(name="ppool", bufs=NT) as ppool,
            tc.tile_pool(name="xpool", bufs=1) as xpool,
            tc.tile_pool(name="small", bufs=2) as spool,
            tc.tile_pool(name="s1part", bufs=2) as s1pool,
            tc.tile_pool(name="mrg", bufs=2) as mpool,
            tc.tile_pool(name="psum", bufs=8, space="PSUM") as psum_pool,
            tc.tile_pool(name="dram", bufs=1, space="DRAM") as dpool,
        ):
            cst = cpool.tile([128, 2 * T], f32, name="cst")
            nc.scalar.dma_start(out=cst[:], in_=consts_ext.ap())

            bar_in = dpool.tile([1, 64], f16, name="bar_in")
            bar_out = dpool.tile([8, 64], f16, name="bar_out")
            nc.gpsimd.collective_compute(
                "AllGather",
                mybir.AluOpType.bypass,
                replica_groups=rg,
                ins=[bar_in.opt()],
                outs=[bar_out.opt()],
            )
            rinv = cst[:, 0:T]
            zero64 = cst[:, T:2 * T]

            stash = {}
            p_tiles = []

            def load_wt(t):
                wt = wpool.tile([128, OUT], f16, name=f"wt{t}", tag="wt", bufs=6)
                nc.sync.dma_start(out=wt[:], in_=w_ext.ap()[128 * t:128 * (t + 1), :])
                stash[("wt", t)] = wt
                if t == 0:
                    xT_sb = xpool.tile([128, NT * B], f16, name="xT_sb")
                    nc.sync.dma_start(out=xT_sb[:], in_=xT_ext.ap())
                    stash["xT"] = xT_sb

            def front(t):
                wt = stash[("wt", t)]
                # DVE: candidate scan + sort
                cand = spool.tile([128, NCAND], f16, tag="cand", bufs=2)
                for c in range(8):
                    nc.vector.max(cand[:, 8 * c:8 * c + 8], wt[:, 512 * c:512 * (c + 1)])
                v64 = spool.tile([128, T], f16, tag="v64", bufs=2)
                for j in range(T // 8):
                    nc.vector.max(v64[:, 8 * j:8 * j + 8], cand[:])
                    if j < T // 8 - 1:
                        nc.vector.match_replace(
                            cand[:], v64[:, 8 * j:8 * j + 8], cand[:], NEG_FILL
                        )
                # GpSimd: the whole threshold chain (small [128,T] ops)
                m32 = spool.tile([128, 1], f32, tag="m32", bufs=2)
                nc.gpsimd.tensor_copy(m32[:], v64[:, 0:1])
                zs = spool.tile([128, T], f32, tag="zs", bufs=2)
                nc.vector.tensor_scalar(
                    zs[:], v64[:], m32[:], 0.5, AluOpType.subtract, AluOpType.mult
                )
                zsq = spool.tile([128, T], f32, tag="zsq", bufs=2)
                nc.vector.tensor_tensor(zsq[:], zs[:], zs[:], AluOpType.mult)
                cs1 = spool.tile([128, T], f32, tag="cs1", bufs=2)
                nc.vector.tensor_tensor_scan(
                    cs1[:], zs[:], zero64, 0.0, AluOpType.add, AluOpType.add
                )
                cs2 = spool.tile([128, T], f32, tag="cs2", bufs=2)
                nc.vector.tensor_tensor_scan(
                    cs2[:], zsq[:], zero64, 0.0, AluOpType.add, AluOpType.add
                )
                mean = spool.tile([128, T], f32, tag="mean", bufs=2)
                nc.gpsimd.tensor_tensor(mean[:], cs1[:], rinv, AluOpType.mult)
                msq = spool.tile([128, T], f32, tag="msq", bufs=2)
                nc.gpsimd.tensor_tensor(msq[:], cs2[:], rinv, AluOpType.mult)
                ms2 = spool.tile([128, T], f32, tag="ms2", bufs=2)
                nc.gpsimd.tensor_tensor(ms2[:], mean[:], mean[:], AluOpType.mult)
                dta = spool.tile([128, T], f32, tag="dta", bufs=2)
                nc.gpsimd.tensor_tensor(dta[:], rinv, msq[:], AluOpType.subtract)
                nc.gpsimd.tensor_tensor(dta[:], dta[:], ms2[:], AluOpType.add)
                nc.gpsimd.tensor_single_scalar(dta[:], dta[:], 0.0, AluOpType.max)
                sq = spool.tile([128, T], f32, tag="sq", bufs=2)
                nc.scalar.activation(sq[:], dta[:], mybir.ActivationFunctionType.Sqrt)
                stash[t] = (wt, m32, zs, mean, sq)

            def back(t):
                wt, m32, zs, mean, sq = stash.pop(t)
                # GpSimd: tau..cneg (stay on one engine to cut hops)
                tau = spool.tile([128, T], f32, tag="tau", bufs=2)
                nc.gpsimd.tensor_tensor(tau[:], mean[:], sq[:], AluOpType.subtract)
                # tsel = tau+100 where tau<=zs else pushed far below:
                # f = max(-1e6*(zs-tau), 0); tsel = (tau+100) - f
                # (avoids is_le, which the Pool engine lacks; boundary
                # margins <1e-4 are support-edge ties with negligible effect)
                dd = spool.tile([128, T], f32, tag="dd", bufs=2)
                nc.gpsimd.tensor_tensor(dd[:], zs[:], tau[:], AluOpType.subtract)
                nc.gpsimd.tensor_single_scalar(dd[:], dd[:], -1e6,
                                               AluOpType.mult)
                nc.gpsimd.tensor_single_scalar(dd[:], dd[:], 0.0,
                                               AluOpType.max)
                tsel = spool.tile([128, T], f32, tag="tsel", bufs=2)
                nc.gpsimd.tensor_single_scalar(tsel[:], tau[:], 100.0,
                                               AluOpType.add)
                nc.gpsimd.tensor_tensor(tsel[:], tsel[:], dd[:],
                                        AluOpType.subtract)
                tmax = spool.tile([128, 1], f32, tag="tmax", bufs=2)
                nc.vector.tensor_reduce(
                    tmax[:], tsel[:], mybir.AxisListType.X, AluOpType.max
                )
                c1 = spool.tile([128, 1], f32, tag="c1", bufs=2)
                nc.gpsimd.tensor_single_scalar(c1[:], tmax[:], -2.0,
                                               AluOpType.mult)
                nc.gpsimd.tensor_single_scalar(c1[:], c1[:], 200.0,
                                               AluOpType.add)
                cneg = spool.tile([128, 1], f32, tag="cneg", bufs=2)
                nc.gpsimd.tensor_tensor(cneg[:], c1[:], m32[:], AluOpType.subtract)
                r = spool.tile([128, OUT], f16, tag="r", bufs=2, name=f"r{t}")
                nc.scalar.activation(
                    r[:], wt[:], mybir.ActivationFunctionType.Relu,
                    bias=cneg[:], scale=1.0,
                )
                p = ppool.tile([128, OUT], f16, tag="p", name=f"p{t}")
                if t in (1, 4):
                    nc.vector.tensor_tensor(p[:], r[:], r[:], AluOpType.mult)
                else:
                    nc.scalar.activation(
                        p[:], r[:], mybir.ActivationFunctionType.Square
                    )
                p_tiles.append(p)

            def mm_session(b, t0, t1, ps_tiles):
                xT_sb = stash["xT"]
                for t in range(t0, t1):
                    for kq in range(8):
                        nc.tensor.matmul(
                            ps_tiles[kq][:],
                            lhsT=xT_sb[:, 512 * t + 128 * b:512 * t + 128 * (b + 1)],
                            rhs=p_tiles[t][:, 512 * kq:512 * (kq + 1)],
                            start=(t == t0),
                            stop=(t == t1 - 1),
                        )

            def evac_s1(b, ps_tiles, s1parts):
                s1p = s1pool.tile([128, OUT], f16, tag="s1p", bufs=2,
                                  name=f"s1p{b}")
                for kq in range(8):
                    dst = s1p[:, 512 * kq:512 * (kq + 1)]
                    if kq % 2 == 0:
                        nc.vector.tensor_copy(dst, ps_tiles[kq][:])
                    else:
                        nc.scalar.copy(dst, ps_tiles[kq][:])
                s1parts[b] = s1p

            def merge_s2(b, ps_tiles, s1parts, partial):
                # psum(s2) + s1p -> f16, kq-pipelined; halves DMA'd as ready
                s1p = s1parts[b]
                out16 = mpool.tile([128, OUT], f16, tag="mrg", bufs=2,
                                   name=f"mrg{b}")
                tmp = mpool.tile([128, OUT // 2], f16, tag="mtmp", bufs=2,
                                 name=f"mtmp{b}")
                for kq in range(8):
                    if kq % 2 == 0:
                        nc.vector.scalar_tensor_tensor(
                            out16[:, 512 * kq:512 * (kq + 1)],
                            ps_tiles[kq][:], 1.0,
                            s1p[:, 512 * kq:512 * (kq + 1)],
                            AluOpType.mult, AluOpType.add,
                        )
                    else:
                        j = (kq - 1) // 2
                        nc.scalar.copy(tmp[:, 512 * j:512 * (j + 1)],
                                       ps_tiles[kq][:, 0:512])
                        nc.vector.tensor_tensor(
                            out16[:, 512 * kq:512 * (kq + 1)],
                            tmp[:, 512 * j:512 * (j + 1)],
                            s1p[:, 512 * kq:512 * (kq + 1)],
                            AluOpType.add,
                        )
                    if kq == 3:
                        nc.sync.dma_start(out=partial[:, 0:2048],
                                          in_=out16[:, 0:2048])
                nc.sync.dma_start(out=partial[:, 2048:OUT],
                                  in_=out16[:, 2048:OUT])

            def evac_full(b, ps_tiles, partial):
                # full-chain blocks: psum -> f16 -> DRAM (DVE/Act split)
                out16 = mpool.tile([128, OUT], f16, tag="mrg", bufs=2,
                                   name=f"mrgf{b}")
                for kq in range(8):
                    dst = out16[:, 512 * kq:512 * (kq + 1)]
                    if kq % 2 == 0:
                        nc.vector.tensor_copy(dst, ps_tiles[kq][:])
                    else:
                        nc.scalar.copy(dst, ps_tiles[kq][:])
                    if kq == 3:
                        nc.sync.dma_start(out=partial[:, 0:2048],
                                          in_=out16[:, 0:2048])
                nc.sync.dma_start(out=partial[:, 2048:OUT],
                                  in_=out16[:, 2048:OUT])

            def rs_chunk(parts, blocks, name):
                ncols_rows = 128 * len(blocks)
                rsout = dpool.tile([16 * len(blocks), OUT], f16,
                                   name=f"rsout{name}")
                nc.gpsimd.collective_compute(
                    "ReduceScatter",
                    mybir.AluOpType.add,
                    replica_groups=rg,
                    ins=[parts.opt()],
                    outs=[rsout.opt()],
                )
                return rsout

            # ---------------- emission ----------------
            for t in range(NT):
                load_wt(t)

            s1parts = {}
            # b0+b1 partials contiguous? separate DRAM tiles; b2+b3 share one
            part0 = dpool.tile([128, OUT], f16, name="partial0")
            part1 = dpool.tile([128, OUT], f16, name="partial1")
            part23 = dpool.tile([256, OUT], f16, name="partial23")

            ps_s1 = {}
            for t in range(NT):
                front(t)
                if t >= 1:
                    back(t - 1)
                if t in (4, 5):
                    b = t - 4
                    ps = [psum_pool.tile([128, 512], f32, tag="ps",
                                         name=f"ps1_{b}_{kq}")
                          for kq in range(8)]
                    ps_s1[b] = ps
                    mm_session(b, 0, S1_TILES, ps)
                    evac_s1(b, ps, s1parts)
            back(NT - 1)

            # post-p7: b0.s2, b1.s2, b2 full, b3 full; RS chunks interleaved
            ps = [psum_pool.tile([128, 512], f32, tag="ps", name=f"ps2_0_{kq}")
                  for kq in range(8)]
            mm_session(0, S1_TILES, NT, ps)
            merge_s2(0, ps, s1parts, part0)
            rs0 = rs_chunk(part0, [0], "0")

            ps = [psum_pool.tile([128, 512], f32, tag="ps", name=f"ps2_1_{kq}")
                  for kq in range(8)]
            mm_session(1, S1_TILES, NT, ps)
            merge_s2(1, ps, s1parts, part1)
            rs1 = rs_chunk(part1, [1], "1")

            ps = [psum_pool.tile([128, 512], f32, tag="ps", name=f"psf_2_{kq}")
                  for kq in range(8)]
            mm_session(2, 0, NT, ps)
            evac_full(2, ps, part23[0:128, :])

            nc.scalar.dma_start(out=out_ext.ap()[0:16, :], in_=rs0[:])

            ps = [psum_pool.tile([128, 512], f32, tag="ps", name=f"psf_3_{kq}")
                  for kq in range(8)]
            mm_session(3, 0, NT, ps)
            evac_full(3, ps, part23[128:256, :])
            rs23 = rs_chunk(part23, [2, 3], "23")

            nc.scalar.dma_start(out=out_ext.ap()[16:32, :], in_=rs1[:])
            nc.scalar.dma_start(out=out_ext.ap()[32:64, :], in_=rs23[:])

    nc.compile()
    return nc


def get_program():
    if "nc" not in _cache:
        _cache["nc"] = _build_program()
    return _cache["nc"]


def kernel(x: np.ndarray, weight: np.ndarray, trace: bool = False):
    from concourse.bass_utils import run_bass_kernel_spmd

    x = np.ascontiguousarray(x, dtype=np.float32)
    weight = np.ascontiguousarray(weight, dtype=np.float32)
    assert x.shape == (B, IN) and weight.shape == (IN, OUT)

    nc = get_program()
    in_maps = []
    for d in range(NCORES):
        wsh = np.ascontiguousarray(
            weight[ROWS * d:ROWS * (d + 1), :], dtype=np.float16
        )
        xsh = (0.25 * x[:, ROWS * d:ROWS * (d + 1)].T).astype(np.float16)
        xt = np.ascontiguousarray(
            xsh.reshape(NT, 128, B).transpose(1, 0, 2).reshape(128, NT * B)
        )
        rho = np.arange(1, T + 1, dtype=np.float32)
        cst = np.zeros((128, 2 * T), dtype=np.float32)
        cst[:, 0:T] = 1.0 / rho
        in_maps.append({"w": wsh, "xT": xt, "consts": cst})
    res = run_bass_kernel_spmd(
        nc, in_maps, core_ids=list(range(NCORES)), trace=trace
    )
    out = np.empty((B, OUT), dtype=np.float32)
    for d in range(NCORES):
        o = res.results[d]["out"].astype(np.float32)  # [64, 4096]
        # rows 0:16 = block b0 slice d; 16:32 = b1 slice d;
        # 32:64 = contiguous 32-row chunk d of the combined [b2;b3] input
        out[16 * d:16 * (d + 1), :] = o[0:16, :]
        out[128 + 16 * d:128 + 16 * (d + 1), :] = o[16:32, :]
        out[256 + 32 * d:256 + 32 * (d + 1), :] = o[32:64, :]
    if trace:
        _cache["last_result"] = res
    return out
